# revision 18
# baseline (speedup 1.0000x reference)
"""Trainium2 Bass kernel for nn_Capsule (dynamic routing, 3 iterations).

Strategy (data-parallel over batch, 8 cores x 16 batch items):
The reference computes u_hat = u_vecs @ W (268 MB) and then routes over it.
We never materialize u_hat.  With Wr = W.reshape(C, N, D):

    s_k[b]   = c_k[b] @ u_vecs[b]                  (N, C)   contraction over i
    M_k[b]   = einsum('nc,cnd->nd', s_k, Wr)       (N, D)   tiny
    out_k[b] = squash(M_k[b])
    t_k[b]   = einsum('nd,cnd->nc', out_k, Wr)     (N, C)   tiny
    b_{k+1}[b] = u_vecs[b] @ t_k[b].T              (I, N)   contraction over c

Iteration 1 has uniform c (b=0), so s_1/out_1/t_1 are data-independent
reductions precomputed on the host.  The device runs two rounds:
  round A: b_2 = uv @ t1.T -> softmax -> s_2 -> M_2 -> out_2 -> t_2
  round B: b_3 = uv @ t2.T -> softmax -> s_3 -> M_3 -> out_3 -> store

b_logits come out as [i on partitions, n on free] so the softmax over n is a
free-dim op (no max subtraction needed; logits are O(+-4)).  All big PE
operands are bf16 (u_vecs streamed as moving operand for s; u_vecsT loaded as
stationary weights for the b-update); the small W-side matmuls stay fp32.
"""

import os
import sys
from contextlib import ExitStack

import numpy as np

for _p in (
    "/root/.axon_site",
    "/root/.axon_site/_ro/trn_rl_repo",
    "/root/.axon_site/_ro/pypackages",
):
    if os.path.isdir(_p) and _p not in sys.path:
        sys.path.append(_p)

import ml_dtypes  # noqa: E402
import concourse.bass as bass  # noqa: E402
import concourse.mybir as mybir  # noqa: E402
import concourse.tile as tile  # noqa: E402
from concourse.bass_utils import run_bass_kernel_spmd  # noqa: E402
from concourse.vector_clock import ScopedClock  # noqa: E402

# ---------------------------------------------------------------------------
# Workaround: TileContext's final drain accumulates >1 sem-waits on a single
# instruction; this walrus build rejects multi-wait instructions ("Too many
# sync wait commands").  Split them into single-wait NoOps on SP.
# ---------------------------------------------------------------------------


def _split_drain_and_barrier(self, tick_clock, wait_clock):
    probe = mybir.InstNoOp(
        name=f"tile-final-wait-probe-{self.nc.next_id()}",
        engine=mybir.EngineType.SP,
    )
    wait_clock.add_sem_waits(probe, ScopedClock({None: tick_clock.global_clock}))
    waits = list(probe.sync_info.on_wait) if probe.sync_info is not None else []
    for w in waits:
        nop = mybir.InstNoOp(
            name=f"tile-final-wait-{self.nc.next_id()}",
            engine=mybir.EngineType.SP,
            sync_info=mybir.SyncInfo(on_wait=[w], on_update=[]),
            bass_nofuse=True,
        )
        self._add_instruction(nop)
    self.nc.sync.drain()
    self.nc.all_engine_barrier()
    popped = self.nc._tile_sem_poison_stack.pop()
    assert popped is self._sem_poison
    self.nc.clear_and_free_semaphores(list(self.sems.allocated().values()))
    self.nc.all_engine_barrier()


tile.TileContext._drain_and_barrier = _split_drain_and_barrier


def _split_multi_waits(nc):
    """Walrus build rejects instructions with >1 sync wait; hoist extras into
    single-wait NoOps on the same engine immediately before the instruction."""
    for f in nc.m.functions:
        for bb in f.blocks:
            new = []
            changed = False
            for inst in bb.instructions:
                si = inst.sync_info
                if si is not None and len(si.on_wait) > 1:
                    waits = list(si.on_wait)
                    for w in waits[:-1]:
                        new.append(
                            mybir.InstNoOp(
                                name=f"wsplit-{nc.next_id()}",
                                engine=inst.engine,
                                sync_info=mybir.SyncInfo(on_wait=[w], on_update=[]),
                                bass_nofuse=True,
                            )
                        )
                    inst.sync_info = mybir.SyncInfo(
                        on_wait=[waits[-1]], on_update=list(si.on_update)
                    )
                    changed = True
                new.append(inst)
            if changed:
                bb.instructions = new

# ---------------------------------------------------------------------------
# Problem constants (hardcoded; kernel.py must be self-contained).
# ---------------------------------------------------------------------------
B, I, C = 128, 1024, 256  # batch, in_caps, in_dim
N, D = 16, 32  # num_capsule, dim_capsule
ND = N * D  # 512
EPS = 1e-7
CORES = 8
BL = B // CORES  # 16 batch items per core
IT = I // 128  # 8 i-tiles
CT = C // 128  # 2 c-tiles
MT = ND // 128  # 4 (n,d)-tiles

_f32 = mybir.dt.float32
_bf16 = mybir.dt.bfloat16
_EXP = mybir.ActivationFunctionType.Exp
_SQUARE = mybir.ActivationFunctionType.Square
_SQRT = mybir.ActivationFunctionType.Sqrt
_ADD = mybir.AluOpType.add
_AXX = mybir.AxisListType.X

_CACHE = {}

# Derive u_vecsT on-device via xbar DMA transposes (halves HBM traffic).
ONDEV_T = False


def _build_nc():
    nc = bass.Bass()
    uv_d = nc.dram_tensor("uv", (BL, 128, IT * C), _bf16, kind="ExternalInput")
    if not ONDEV_T:
        uvT_d = nc.dram_tensor("uvT", (BL, 128, CT * I), _bf16, kind="ExternalInput")
    t1T_d = nc.dram_tensor("t1T", (128, CT, N * BL), _bf16, kind="ExternalInput")
    Wm_d = nc.dram_tensor("Wm", (128, CT, ND), _f32, kind="ExternalInput")
    Wt_d = nc.dram_tensor("Wt", (128, MT, C), _f32, kind="ExternalInput")
    i16_d = nc.dram_tensor("i16", (N, N), _f32, kind="ExternalInput")
    i32_d = nc.dram_tensor("i32", (D, D), _f32, kind="ExternalInput")
    on32_d = nc.dram_tensor("on32", (D, 1), _f32, kind="ExternalInput")
    on1_d = nc.dram_tensor("on1", (1, D), _f32, kind="ExternalInput")
    eps_d = nc.dram_tensor("epsv", (1, 1), _f32, kind="ExternalInput")
    out_d = nc.dram_tensor("out", (BL, ND), _f32, kind="ExternalOutput")

    with tile.TileContext(nc) as tc, ExitStack() as ctx:
        const = ctx.enter_context(tc.tile_pool(name="const", bufs=1))
        uvp = ctx.enter_context(tc.tile_pool(name="uvp", bufs=BL))
        uvtp = ctx.enter_context(tc.tile_pool(name="uvtp", bufs=BL))
        pers = ctx.enter_context(tc.tile_pool(name="pers", bufs=1))
        perb = ctx.enter_context(tc.tile_pool(name="perb", bufs=3))
        psA = ctx.enter_context(tc.tile_pool(name="psA", bufs=3, space="PSUM"))
        psS = ctx.enter_context(tc.tile_pool(name="psS", bufs=3, space="PSUM"))
        psM = ctx.enter_context(tc.tile_pool(name="psM", bufs=1, space="PSUM"))
        psm = ctx.enter_context(tc.tile_pool(name="psm", bufs=1, space="PSUM"))

        # first-needed inputs first: t1T (round-A rhs), then b=0's data
        t1T_sb = const.tile([128, CT, N * BL], _bf16, tag="t1T")
        nc.sync.dma_start(t1T_sb[:], t1T_d[:])

        uv_sb, uvT_sb = [], []

        def load_b(b):
            tv = uvp.tile([128, IT * C], _bf16, tag="uv")
            nc.sync.dma_start(tv[:], uv_d[b])
            uv_sb.append(tv)
            tw = uvtp.tile([128, CT * I], _bf16, tag="uvT")
            if ONDEV_T:
                uvv = tv[:].rearrange("p (t c) -> p t c", t=IT)
                uvTv = tw[:].rearrange("p (s i) -> p s i", s=CT)
                for s in range(CT):
                    for t in range(IT):
                        nc.sync.dma_start_transpose(
                            uvTv[:, s, t * 128 : (t + 1) * 128],
                            uvv[:, t, s * 128 : (s + 1) * 128],
                        )
            else:
                nc.sync.dma_start(tw[:], uvT_d[b])
            uvT_sb.append(tw)

        for b in range(2):
            load_b(b)

        # constants / weights (needed from the s-transposes / mid phase on)
        Wm_sb = const.tile([128, CT, ND], _f32, tag="Wm")
        nc.sync.dma_start(Wm_sb[:], Wm_d[:])
        Wt_sb = const.tile([128, MT, C], _f32, tag="Wt")
        nc.sync.dma_start(Wt_sb[:], Wt_d[:])
        i16_sb = const.tile([N, N], _f32, tag="i16")
        nc.sync.dma_start(i16_sb[:], i16_d[:])
        i32_sb = const.tile([D, D], _f32, tag="i32")
        nc.sync.dma_start(i32_sb[:], i32_d[:])
        on32_sb = const.tile([D, 1], _f32, tag="on32")
        nc.sync.dma_start(on32_sb[:], on32_d[:])
        on1_sb = const.tile([1, D], _f32, tag="on1")
        nc.sync.dma_start(on1_sb[:], on1_d[:])
        eps_sb = const.tile([1, 1], _f32, tag="epsv")
        nc.sync.dma_start(eps_sb[:], eps_d[:])

        for b in range(2, BL):
            load_b(b)

        # Two batch-groups of 8: group 1's DMA streams in while group 0
        # computes, keeping both HBM and PE continuously busy.
        G = 2
        GL = BL // G
        NB = N * GL

        def make_group_tiles(g):
            return {
                "sT": pers.tile([128, CT, NB], _f32, tag=f"sT{g}", name=f"sT{g}"),
                "tT": pers.tile([128, CT, NB], _bf16, tag=f"tT{g}", name=f"tT{g}"),
                "Op": pers.tile([128, MT, NB], _f32, tag=f"Op{g}", name=f"Op{g}"),
                "Md": pers.tile([D, NB], _f32, tag=f"Md{g}", name=f"Md{g}"),
                "Md2": pers.tile([D, NB], _f32, tag=f"Md2{g}", name=f"Md2{g}"),
                "Osq": pers.tile([D, NB], _f32, tag=f"Osq{g}", name=f"Osq{g}"),
                "sq": pers.tile([1, NB], _f32, tag=f"sq{g}", name=f"sq{g}"),
                "inv": pers.tile([1, NB], _f32, tag=f"inv{g}", name=f"inv{g}"),
                "outT": pers.tile([GL, ND], _f32, tag=f"outT{g}", name=f"outT{g}"),
            }

        gts = [make_group_tiles(g) for g in range(G)]

        def bupd(tTv4, lb, b):
            """b_logits[b] = u_vecs[b] @ t.T -> psum [i, n] tile.
            tTv4: [128, CT, N, width] view; lb: column index within it."""
            tTv = tTv4[:, :, :, lb]
            uvTv = uvT_sb[b][:].rearrange("p (s i) -> p s i", s=CT)
            blog = psA.tile([128, IT * N], _f32, tag="blog")
            for ic in range(IT):
                for s in range(CT):
                    nc.tensor.matmul(
                        blog[:, ic * N : (ic + 1) * N],
                        lhsT=uvTv[:, s, ic * 128 : (ic + 1) * 128],
                        rhs=tTv[:, s, :],
                        start=(s == 0),
                        stop=(s == CT - 1),
                    )
            return blog

        def round_perb(g, tTv4, interleave=None):
            """Software-pipelined per-b loop.  bupd(b+1) fills the softmax(b)
            latency on PE; the s->sT tail ops run one iteration late so no
            engine's in-order stream blocks on the current b's chain.
            `interleave`: {lb: closure} extra emissions (mid stages of the
            previous group) placed between iterations."""
            gt = gts[g]
            b0 = g * GL
            blogs = [bupd(tTv4, 0, b0), bupd(tTv4, 1, b0 + 1)]
            pend = {}  # lb -> s_ps tile awaiting tail

            def tail(lb):
                s_ps = pend.pop(lb)
                s_sb = perb.tile([N, C], _f32, tag="ssb", name=f"ssb{g}_{lb}")
                nc.scalar.copy(s_sb[:], s_ps[:])
                sT_ps = psS.tile([128, CT * N], _f32, tag="ssT", name=f"sTp{g}_{lb}")
                for h in range(CT):
                    nc.tensor.transpose(
                        sT_ps[:, h * N : (h + 1) * N],
                        s_sb[:, h * 128 : (h + 1) * 128],
                        i16_sb[:],
                    )
                dst = gt["sT"][:].rearrange("p s (n b) -> p s n b", b=GL)[:, :, :, lb]
                nc.vector.tensor_copy(
                    dst, sT_ps[:].rearrange("p (s n) -> p s n", s=CT)
                )

            for lb in range(GL):
                b = b0 + lb
                if interleave and lb in interleave:
                    interleave[lb]()
                # softmax over n (free dim, groups of 16); no max needed
                blog = blogs[lb]
                expb = perb.tile([128, IT * N], _f32, tag="expb")
                nc.scalar.activation(expb[:], blog[:], _EXP)
                zz = perb.tile([128, IT], _f32, tag="zz")
                nc.vector.tensor_reduce(
                    zz[:],
                    expb[:].rearrange("p (t n) -> p t n", n=N),
                    axis=_AXX,
                    op=_ADD,
                )
                rr = perb.tile([128, IT], _f32, tag="rr")
                nc.vector.reciprocal(rr[:], zz[:])
                cb = perb.tile([128, IT * N], _bf16, tag="cb")
                nc.vector.tensor_mul(
                    cb[:].rearrange("p (t n) -> p t n", n=N),
                    expb[:].rearrange("p (t n) -> p t n", n=N),
                    rr[:].unsqueeze(2).broadcast_to([128, IT, N]),
                )
                if lb + 2 < GL:
                    blogs.append(bupd(tTv4, lb + 2, b + 2))
                # s = c @ u_vecs  (accumulate over i-tiles)
                uvv = uv_sb[b][:].rearrange("p (t c) -> p t c", t=IT)
                s_ps = psS.tile([N, C], _f32, tag="ssT", name=f"sps{g}_{lb}")
                for ic in range(IT):
                    nc.tensor.matmul(
                        s_ps[:],
                        lhsT=cb[:, ic * N : (ic + 1) * N],
                        rhs=uvv[:, ic, :],
                        start=(ic == 0),
                        stop=(ic == IT - 1),
                    )
                pend[lb] = s_ps
                if lb >= 1:
                    tail(lb - 1)
            tail(GL - 1)

        def mid_M(g):
            """M-matmuls + block-diag extraction + square."""
            gt = gts[g]
            for m in range(MT):
                Mp = psM.tile([128, NB], _f32, tag="mid", name=f"Mp{g}_{m}")
                for s in range(CT):
                    nc.tensor.matmul(
                        Mp[:],
                        lhsT=Wm_sb[:, s, m * 128 : (m + 1) * 128],
                        rhs=gt["sT"][:, s, :],
                        start=(s == 0),
                        stop=(s == CT - 1),
                    )
                for q in range(4):
                    n_ = m * 4 + q
                    nc.vector.tensor_copy(
                        gt["Md"][:, n_ * GL : (n_ + 1) * GL],
                        Mp[q * D : (q + 1) * D, n_ * GL : (n_ + 1) * GL],
                    )
            nc.scalar.activation(gt["Md2"][:], gt["Md"][:], _SQUARE)

        def mid_N(g):
            gt = gts[g]
            n2 = psm.tile([1, NB], _f32, tag="sm", name=f"n2{g}")
            nc.tensor.matmul(
                n2[:], lhsT=on32_sb[:], rhs=gt["Md2"][:], start=True, stop=True
            )
            nc.scalar.activation(gt["sq"][:], n2[:], _SQRT, bias=eps_sb[:])
            nc.vector.reciprocal(gt["inv"][:], gt["sq"][:])

        def mid_E(g):
            gt = gts[g]
            inv32 = psm.tile([D, NB], _f32, tag="sm", name=f"i32{g}")
            nc.tensor.matmul(
                inv32[:], lhsT=on1_sb[:], rhs=gt["inv"][:], start=True, stop=True
            )
            nc.vector.tensor_mul(gt["Osq"][:], gt["Md"][:], inv32[:])

        def mid_T(g):
            """outputs -> O' (block-diag) -> t -> tT (bf16)."""
            gt = gts[g]
            for n_ in range(N):
                nc.vector.tensor_copy(
                    gt["Op"][
                        (n_ % 4) * D : (n_ % 4 + 1) * D,
                        n_ // 4,
                        n_ * GL : (n_ + 1) * GL,
                    ],
                    gt["Osq"][:, n_ * GL : (n_ + 1) * GL],
                )
            for cc in range(CT):
                tp = psM.tile([128, NB], _f32, tag="mid", name=f"tp{g}_{cc}")
                for kt in range(MT):
                    nc.tensor.matmul(
                        tp[:],
                        lhsT=Wt_sb[:, kt, cc * 128 : (cc + 1) * 128],
                        rhs=gt["Op"][:, kt, :],
                        start=(kt == 0),
                        stop=(kt == MT - 1),
                    )
                nc.vector.tensor_copy(gt["tT"][:, cc, :], tp[:])

        def mid_Out(g):
            gt = gts[g]
            outT = psm.tile([GL, ND], _f32, tag="sm", name=f"oT{g}")
            for n_ in range(N):
                nc.tensor.transpose(
                    outT[:, n_ * D : (n_ + 1) * D],
                    gt["Osq"][:, n_ * GL : (n_ + 1) * GL],
                    i32_sb[:],
                )
            nc.vector.tensor_copy(gt["outT"][:], outT[:])
            nc.sync.dma_start(out_d[g * GL : (g + 1) * GL], gt["outT"][:])

        t1v = t1T_sb[:].rearrange("p s (n b) -> p s n b", b=BL)
        for g in range(G):
            nc.vector.memset(gts[g]["Op"][:], 0.0)

        def t1view(g):
            return t1v[:, :, :, g * GL : (g + 1) * GL]

        def tTview(g):
            return gts[g]["tT"][:].rearrange("p s (n b) -> p s n b", b=GL)

        # A0 -> A1 -> B0 -> B1, with each mid's serial stages interleaved
        # into the following round so PE never idles on the squash chain.
        round_perb(0, t1view(0))
        mid_M(0)
        round_perb(
            1,
            t1view(1),
            interleave={1: lambda: mid_N(0), 2: lambda: mid_E(0), 3: lambda: mid_T(0)},
        )
        mid_M(1)
        round_perb(
            0,
            tTview(0),
            interleave={1: lambda: mid_N(1), 2: lambda: mid_E(1), 3: lambda: mid_T(1)},
        )
        mid_M(0)
        round_perb(
            1,
            tTview(1),
            interleave={
                1: lambda: mid_N(0),
                2: lambda: mid_E(0),
                3: lambda: mid_Out(0),
            },
        )
        mid_M(1)
        mid_N(1)
        mid_E(1)
        mid_Out(1)

    _split_multi_waits(nc)
    return nc


def _host_prep(u_vecs, W):
    """Shard + reformat inputs; precompute the data-independent iteration 1."""
    f32 = np.float32
    u = np.ascontiguousarray(u_vecs, dtype=f32)
    Wf = np.ascontiguousarray(W, dtype=f32)
    Wr = Wf.reshape(C, N, D)

    # iteration 1 (c uniform): s1 = colsum(u)/N, squash, t1  -- all O(B*I*C)
    s1 = u.sum(axis=1) / N  # (B, C)
    M1 = np.einsum("bc,cnd->bnd", s1, Wr)  # (B, N, D)
    o1 = M1 / np.sqrt((M1 * M1).sum(-1, keepdims=True) + EPS)
    t1 = np.einsum("bnd,cnd->bnc", o1, Wr)  # (B, N, C)

    bf = ml_dtypes.bfloat16
    # uv[b]: [128, it*256] with uv[b, p, t*C+c] = u[b, t*128+p, c]
    uv = np.ascontiguousarray(
        u.reshape(B, IT, 128, C).transpose(0, 2, 1, 3).reshape(B, 128, IT * C)
    ).astype(bf)
    if not ONDEV_T:
        # uvT[b]: [128, s*1024+i] = u[b, i, s*128+p]
        uvT = np.ascontiguousarray(
            u.transpose(0, 2, 1)
            .reshape(B, CT, 128, I)
            .transpose(0, 2, 1, 3)
            .reshape(B, 128, CT * I)
        ).astype(bf)

    shared = {
        "Wm": np.ascontiguousarray(
            Wf.reshape(CT, 128, ND).transpose(1, 0, 2)
        ).astype(f32),
        "Wt": np.ascontiguousarray(
            Wf.T.reshape(MT, 128, C).transpose(1, 0, 2)
        ).astype(f32),
        "i16": np.eye(N, dtype=f32),
        "i32": np.eye(D, dtype=f32),
        "on32": np.ones((D, 1), dtype=f32),
        "on1": np.ones((1, D), dtype=f32),
        "epsv": np.full((1, 1), EPS, dtype=f32),
    }

    in_maps = []
    for core in range(CORES):
        b0 = core * BL
        # t1T[p, s, n*BL+b] = t1[b0+b, n, s*128+p]
        t1T = np.ascontiguousarray(
            t1[b0 : b0 + BL]
            .transpose(2, 1, 0)  # (C, N, BL)
            .reshape(CT, 128, N * BL)
            .transpose(1, 0, 2)
        ).astype(bf)
        m = dict(shared)
        m["uv"] = uv[b0 : b0 + BL]
        if not ONDEV_T:
            m["uvT"] = uvT[b0 : b0 + BL]
        m["t1T"] = t1T
        in_maps.append(m)
    return in_maps


def _run(u_vecs, W, trace=False, **kw):
    if "nc" not in _CACHE:
        _CACHE["nc"] = _build_nc()
    nc = _CACHE["nc"]
    in_maps = _host_prep(u_vecs, W)
    res = run_bass_kernel_spmd(nc, in_maps, core_ids=list(range(CORES)), trace=trace, **kw)
    outs = [res.results[c]["out"] for c in range(CORES)]
    full = np.concatenate(outs, axis=0).reshape(B, N, D).astype(np.float32)
    return full, res


def kernel(u_vecs, W):
    out, _ = _run(u_vecs, W, trace=False)
    return out


# revision 21
# speedup vs baseline: 1.2639x; 1.2639x over previous
"""Trainium2 Bass kernel for nn_Capsule (dynamic routing, 3 iterations).

Strategy (data-parallel over batch, 8 cores x 16 batch items):
The reference computes u_hat = u_vecs @ W (268 MB) and then routes over it.
We never materialize u_hat.  With Wr = W.reshape(C, N, D):

    s_k[b]   = c_k[b] @ u_vecs[b]                  (N, C)   contraction over i
    M_k[b]   = einsum('nc,cnd->nd', s_k, Wr)       (N, D)   tiny
    out_k[b] = squash(M_k[b])
    t_k[b]   = einsum('nd,cnd->nc', out_k, Wr)     (N, C)   tiny
    b_{k+1}[b] = u_vecs[b] @ t_k[b].T              (I, N)   contraction over c

Iteration 1 has uniform c (b=0), so s_1/out_1/t_1 are data-independent
reductions precomputed on the host.  The device runs two rounds:
  round A: b_2 = uv @ t1.T -> softmax -> s_2 -> M_2 -> out_2 -> t_2
  round B: b_3 = uv @ t2.T -> softmax -> s_3 -> M_3 -> out_3 -> store

b_logits come out as [i on partitions, n on free] so the softmax over n is a
free-dim op (no max subtraction needed; logits are O(+-4)).  All big PE
operands are bf16 (u_vecs streamed as moving operand for s; u_vecsT loaded as
stationary weights for the b-update); the small W-side matmuls stay fp32.
"""

import os
import sys
from contextlib import ExitStack

import numpy as np

for _p in (
    "/root/.axon_site",
    "/root/.axon_site/_ro/trn_rl_repo",
    "/root/.axon_site/_ro/pypackages",
):
    if os.path.isdir(_p) and _p not in sys.path:
        sys.path.append(_p)

import ml_dtypes  # noqa: E402
import concourse.bass as bass  # noqa: E402
import concourse.mybir as mybir  # noqa: E402
import concourse.tile as tile  # noqa: E402
from concourse.bass_utils import run_bass_kernel_spmd  # noqa: E402
from concourse.vector_clock import ScopedClock  # noqa: E402

# ---------------------------------------------------------------------------
# Workaround: TileContext's final drain accumulates >1 sem-waits on a single
# instruction; this walrus build rejects multi-wait instructions ("Too many
# sync wait commands").  Split them into single-wait NoOps on SP.
# ---------------------------------------------------------------------------


def _split_drain_and_barrier(self, tick_clock, wait_clock):
    probe = mybir.InstNoOp(
        name=f"tile-final-wait-probe-{self.nc.next_id()}",
        engine=mybir.EngineType.SP,
    )
    wait_clock.add_sem_waits(probe, ScopedClock({None: tick_clock.global_clock}))
    waits = list(probe.sync_info.on_wait) if probe.sync_info is not None else []
    for w in waits:
        nop = mybir.InstNoOp(
            name=f"tile-final-wait-{self.nc.next_id()}",
            engine=mybir.EngineType.SP,
            sync_info=mybir.SyncInfo(on_wait=[w], on_update=[]),
            bass_nofuse=True,
        )
        self._add_instruction(nop)
    self.nc.sync.drain()
    self.nc.all_engine_barrier()
    popped = self.nc._tile_sem_poison_stack.pop()
    assert popped is self._sem_poison
    self.nc.clear_and_free_semaphores(list(self.sems.allocated().values()))
    self.nc.all_engine_barrier()


tile.TileContext._drain_and_barrier = _split_drain_and_barrier


def _split_multi_waits(nc):
    """Walrus build rejects instructions with >1 sync wait; hoist extras into
    single-wait NoOps on the same engine immediately before the instruction."""
    for f in nc.m.functions:
        for bb in f.blocks:
            new = []
            changed = False
            for inst in bb.instructions:
                si = inst.sync_info
                if si is not None and len(si.on_wait) > 1:
                    waits = list(si.on_wait)
                    for w in waits[:-1]:
                        new.append(
                            mybir.InstNoOp(
                                name=f"wsplit-{nc.next_id()}",
                                engine=inst.engine,
                                sync_info=mybir.SyncInfo(on_wait=[w], on_update=[]),
                                bass_nofuse=True,
                            )
                        )
                    inst.sync_info = mybir.SyncInfo(
                        on_wait=[waits[-1]], on_update=list(si.on_update)
                    )
                    changed = True
                new.append(inst)
            if changed:
                bb.instructions = new

# ---------------------------------------------------------------------------
# Problem constants (hardcoded; kernel.py must be self-contained).
# ---------------------------------------------------------------------------
B, I, C = 128, 1024, 256  # batch, in_caps, in_dim
N, D = 16, 32  # num_capsule, dim_capsule
ND = N * D  # 512
EPS = 1e-7
CORES = 8
BL = B // CORES  # 16 batch items per core
IT = I // 128  # 8 i-tiles
CT = C // 128  # 2 c-tiles
MT = ND // 128  # 4 (n,d)-tiles

_f32 = mybir.dt.float32
_bf16 = mybir.dt.bfloat16
_EXP = mybir.ActivationFunctionType.Exp
_SQUARE = mybir.ActivationFunctionType.Square
_SQRT = mybir.ActivationFunctionType.Sqrt
_ADD = mybir.AluOpType.add
_AXX = mybir.AxisListType.X

_CACHE = {}

# Derive u_vecsT on-device via xbar DMA transposes (halves HBM traffic).
ONDEV_T = False


def _build_nc():
    nc = bass.Bass()
    uv_d = nc.dram_tensor("uv", (BL, 128, IT * C), _bf16, kind="ExternalInput")
    if not ONDEV_T:
        uvT_d = nc.dram_tensor("uvT", (BL, 128, CT * I), _bf16, kind="ExternalInput")
    t1T_d = nc.dram_tensor("t1T", (128, CT, N * BL), _bf16, kind="ExternalInput")
    Wm_d = nc.dram_tensor("Wm", (128, CT, ND), _f32, kind="ExternalInput")
    Wt_d = nc.dram_tensor("Wt", (128, MT, C), _f32, kind="ExternalInput")
    i16_d = nc.dram_tensor("i16", (N, N), _f32, kind="ExternalInput")
    i32_d = nc.dram_tensor("i32", (D, D), _f32, kind="ExternalInput")
    on32_d = nc.dram_tensor("on32", (D, 1), _f32, kind="ExternalInput")
    on1_d = nc.dram_tensor("on1", (1, D), _f32, kind="ExternalInput")
    eps_d = nc.dram_tensor("epsv", (1, 1), _f32, kind="ExternalInput")
    out_d = nc.dram_tensor("out", (BL, ND), _f32, kind="ExternalOutput")

    with tile.TileContext(nc) as tc, ExitStack() as ctx:
        const = ctx.enter_context(tc.tile_pool(name="const", bufs=1))
        uvp = ctx.enter_context(tc.tile_pool(name="uvp", bufs=BL))
        uvtp = ctx.enter_context(tc.tile_pool(name="uvtp", bufs=BL))
        pers = ctx.enter_context(tc.tile_pool(name="pers", bufs=1))
        perb = ctx.enter_context(tc.tile_pool(name="perb", bufs=3))
        psA = ctx.enter_context(tc.tile_pool(name="psA", bufs=3, space="PSUM"))
        psS = ctx.enter_context(tc.tile_pool(name="psS", bufs=2, space="PSUM"))
        psM = ctx.enter_context(tc.tile_pool(name="psM", bufs=2, space="PSUM"))
        psm = ctx.enter_context(tc.tile_pool(name="psm", bufs=1, space="PSUM"))

        # first-needed inputs first: t1T (round-A rhs), then b=0's data
        t1T_sb = const.tile([128, CT, N * BL], _bf16, tag="t1T")
        nc.sync.dma_start(t1T_sb[:], t1T_d[:])

        uv_sb, uvT_sb = [], []

        def load_b(b):
            tv = uvp.tile([128, IT * C], _bf16, tag="uv")
            nc.sync.dma_start(tv[:], uv_d[b])
            uv_sb.append(tv)
            tw = uvtp.tile([128, CT * I], _bf16, tag="uvT")
            if ONDEV_T:
                uvv = tv[:].rearrange("p (t c) -> p t c", t=IT)
                uvTv = tw[:].rearrange("p (s i) -> p s i", s=CT)
                for s in range(CT):
                    for t in range(IT):
                        nc.sync.dma_start_transpose(
                            uvTv[:, s, t * 128 : (t + 1) * 128],
                            uvv[:, t, s * 128 : (s + 1) * 128],
                        )
            else:
                nc.sync.dma_start(tw[:], uvT_d[b])
            uvT_sb.append(tw)

        for b in range(2):
            load_b(b)

        # constants / weights (needed from the s-transposes / mid phase on)
        Wm_sb = const.tile([128, CT, ND], _f32, tag="Wm")
        nc.sync.dma_start(Wm_sb[:], Wm_d[:])
        Wt_sb = const.tile([128, MT, C], _f32, tag="Wt")
        nc.sync.dma_start(Wt_sb[:], Wt_d[:])
        i16_sb = const.tile([N, N], _f32, tag="i16")
        nc.sync.dma_start(i16_sb[:], i16_d[:])
        i32_sb = const.tile([D, D], _f32, tag="i32")
        nc.sync.dma_start(i32_sb[:], i32_d[:])
        on32_sb = const.tile([D, 1], _f32, tag="on32")
        nc.sync.dma_start(on32_sb[:], on32_d[:])
        on1_sb = const.tile([1, D], _f32, tag="on1")
        nc.sync.dma_start(on1_sb[:], on1_d[:])
        eps_sb = const.tile([1, 1], _f32, tag="epsv")
        nc.sync.dma_start(eps_sb[:], eps_d[:])

        for b in range(2, BL):
            load_b(b)

        # Two batch-groups of 8: group 1's DMA streams in while group 0
        # computes, keeping both HBM and PE continuously busy.
        G = 2
        GL = BL // G
        NB = N * GL

        def make_group_tiles(g):
            return {
                "sT": pers.tile([128, CT, NB], _f32, tag=f"sT{g}", name=f"sT{g}"),
                "tT": pers.tile([128, CT, NB], _bf16, tag=f"tT{g}", name=f"tT{g}"),
                "Op": pers.tile([128, MT, NB], _f32, tag=f"Op{g}", name=f"Op{g}"),
                "Md": pers.tile([D, NB], _f32, tag=f"Md{g}", name=f"Md{g}"),
                "Md2": pers.tile([D, NB], _f32, tag=f"Md2{g}", name=f"Md2{g}"),
                "Osq": pers.tile([D, NB], _f32, tag=f"Osq{g}", name=f"Osq{g}"),
                "sq": pers.tile([1, NB], _f32, tag=f"sq{g}", name=f"sq{g}"),
                "inv": pers.tile([1, NB], _f32, tag=f"inv{g}", name=f"inv{g}"),
                "outT": pers.tile([GL, ND], _f32, tag=f"outT{g}", name=f"outT{g}"),
            }

        gts = [make_group_tiles(g) for g in range(G)]

        def bupd(tTv4, lb, b):
            """b_logits[b] = u_vecs[b] @ t.T -> psum [i, n] tile.
            tTv4: [128, CT, N, width] view; lb: column index within it."""
            tTv = tTv4[:, :, :, lb]
            uvTv = uvT_sb[b][:].rearrange("p (s i) -> p s i", s=CT)
            blog = psA.tile([128, IT * N], _f32, tag="blog")
            for ic in range(IT):
                for s in range(CT):
                    nc.tensor.matmul(
                        blog[:, ic * N : (ic + 1) * N],
                        lhsT=uvTv[:, s, ic * 128 : (ic + 1) * 128],
                        rhs=tTv[:, s, :],
                        start=(s == 0),
                        stop=(s == CT - 1),
                    )
            return blog

        def round_perb(g, tTv4, interleave=None):
            """Software-pipelined per-b loop.  bupd(b+1) fills the softmax(b)
            latency on PE; the s->sT tail ops run one iteration late so no
            engine's in-order stream blocks on the current b's chain.
            `interleave`: {lb: closure} extra emissions (mid stages of the
            previous group) placed between iterations."""
            gt = gts[g]
            b0 = g * GL
            blogs = [bupd(tTv4, 0, b0), bupd(tTv4, 1, b0 + 1)]
            pend = {}  # lb -> s_ps tile awaiting tail

            def tail(lb):
                sT_ps = pend.pop(lb)
                dst = gt["sT"][:].rearrange("p s (n b) -> p s n b", b=GL)[:, :, :, lb]
                nc.vector.tensor_copy(
                    dst, sT_ps[:].rearrange("p (s n) -> p s n", s=CT)
                )

            for lb in range(GL):
                b = b0 + lb
                if interleave and lb in interleave:
                    interleave[lb]()
                # softmax over n (free dim, groups of 16); no max needed
                blog = blogs[lb]
                expb = perb.tile([128, IT * N], _f32, tag="expb")
                nc.scalar.activation(expb[:], blog[:], _EXP)
                zz = perb.tile([128, IT], _f32, tag="zz")
                nc.vector.tensor_reduce(
                    zz[:],
                    expb[:].rearrange("p (t n) -> p t n", n=N),
                    axis=_AXX,
                    op=_ADD,
                )
                rr = perb.tile([128, IT], _f32, tag="rr")
                nc.vector.reciprocal(rr[:], zz[:])
                cb = perb.tile([128, IT * N], _bf16, tag="cb")
                nc.vector.tensor_mul(
                    cb[:].rearrange("p (t n) -> p t n", n=N),
                    expb[:].rearrange("p (t n) -> p t n", n=N),
                    rr[:].unsqueeze(2).broadcast_to([128, IT, N]),
                )
                if lb + 2 < GL:
                    blogs.append(bupd(tTv4, lb + 2, b + 2))
                # sT = (c @ u_vecs).T computed directly: uv tiles stationary,
                # c slices moving -> lands [c on partitions, n free], no
                # transpose / psum round-trip needed.
                uvv = uv_sb[b][:].rearrange("p (t c) -> p t c", t=IT)
                sT_ps = psS.tile([128, CT * N], _f32, tag="ssT", name=f"sps{g}_{lb}")
                for cc in range(CT):
                    for ic in range(IT):
                        nc.tensor.matmul(
                            sT_ps[:, cc * N : (cc + 1) * N],
                            lhsT=uvv[:, ic, cc * 128 : (cc + 1) * 128],
                            rhs=cb[:, ic * N : (ic + 1) * N],
                            start=(ic == 0),
                            stop=(ic == IT - 1),
                        )
                pend[lb] = sT_ps
                if lb >= 1:
                    tail(lb - 1)
            tail(GL - 1)

        def mid_M(g):
            """M-matmuls + block-diag extraction + square."""
            gt = gts[g]
            for m in range(MT):
                Mp = psM.tile([128, NB], _f32, tag="mid", name=f"Mp{g}_{m}")
                for s in range(CT):
                    nc.tensor.matmul(
                        Mp[:],
                        lhsT=Wm_sb[:, s, m * 128 : (m + 1) * 128],
                        rhs=gt["sT"][:, s, :],
                        start=(s == 0),
                        stop=(s == CT - 1),
                    )
                for q in range(4):
                    n_ = m * 4 + q
                    nc.vector.tensor_copy(
                        gt["Md"][:, n_ * GL : (n_ + 1) * GL],
                        Mp[q * D : (q + 1) * D, n_ * GL : (n_ + 1) * GL],
                    )
            nc.scalar.activation(gt["Md2"][:], gt["Md"][:], _SQUARE)

        def mid_N(g):
            gt = gts[g]
            n2 = psm.tile([1, NB], _f32, tag="sm", name=f"n2{g}")
            nc.tensor.matmul(
                n2[:], lhsT=on32_sb[:], rhs=gt["Md2"][:], start=True, stop=True
            )
            nc.scalar.activation(gt["sq"][:], n2[:], _SQRT, bias=eps_sb[:])
            nc.vector.reciprocal(gt["inv"][:], gt["sq"][:])

        def mid_E(g):
            gt = gts[g]
            inv32 = psm.tile([D, NB], _f32, tag="sm", name=f"i32{g}")
            nc.tensor.matmul(
                inv32[:], lhsT=on1_sb[:], rhs=gt["inv"][:], start=True, stop=True
            )
            nc.vector.tensor_mul(gt["Osq"][:], gt["Md"][:], inv32[:])

        def mid_T(g):
            """outputs -> O' (block-diag) -> t -> tT (bf16)."""
            gt = gts[g]
            for n_ in range(N):
                nc.vector.tensor_copy(
                    gt["Op"][
                        (n_ % 4) * D : (n_ % 4 + 1) * D,
                        n_ // 4,
                        n_ * GL : (n_ + 1) * GL,
                    ],
                    gt["Osq"][:, n_ * GL : (n_ + 1) * GL],
                )
            for cc in range(CT):
                tp = psM.tile([128, NB], _f32, tag="mid", name=f"tp{g}_{cc}")
                for kt in range(MT):
                    nc.tensor.matmul(
                        tp[:],
                        lhsT=Wt_sb[:, kt, cc * 128 : (cc + 1) * 128],
                        rhs=gt["Op"][:, kt, :],
                        start=(kt == 0),
                        stop=(kt == MT - 1),
                    )
                nc.vector.tensor_copy(gt["tT"][:, cc, :], tp[:])

        def mid_Out(g):
            gt = gts[g]
            outT = psm.tile([GL, ND], _f32, tag="sm", name=f"oT{g}")
            for n_ in range(N):
                nc.tensor.transpose(
                    outT[:, n_ * D : (n_ + 1) * D],
                    gt["Osq"][:, n_ * GL : (n_ + 1) * GL],
                    i32_sb[:],
                )
            nc.vector.tensor_copy(gt["outT"][:], outT[:])
            nc.sync.dma_start(out_d[g * GL : (g + 1) * GL], gt["outT"][:])

        t1v = t1T_sb[:].rearrange("p s (n b) -> p s n b", b=BL)
        for g in range(G):
            nc.vector.memset(gts[g]["Op"][:], 0.0)

        def t1view(g):
            return t1v[:, :, :, g * GL : (g + 1) * GL]

        def tTview(g):
            return gts[g]["tT"][:].rearrange("p s (n b) -> p s n b", b=GL)

        # A0 -> A1 -> B0 -> B1, with each mid's serial stages interleaved
        # into the following round so PE never idles on the squash chain.
        round_perb(0, t1view(0))
        mid_M(0)
        round_perb(
            1,
            t1view(1),
            interleave={1: lambda: mid_N(0), 2: lambda: mid_E(0), 3: lambda: mid_T(0)},
        )
        mid_M(1)
        round_perb(
            0,
            tTview(0),
            interleave={1: lambda: mid_N(1), 2: lambda: mid_E(1), 3: lambda: mid_T(1)},
        )
        mid_M(0)
        round_perb(
            1,
            tTview(1),
            interleave={
                1: lambda: mid_N(0),
                2: lambda: mid_E(0),
                3: lambda: mid_Out(0),
            },
        )
        mid_M(1)
        mid_N(1)
        mid_E(1)
        mid_Out(1)

    _split_multi_waits(nc)
    return nc


def _host_prep(u_vecs, W):
    """Shard + reformat inputs; precompute the data-independent iteration 1."""
    f32 = np.float32
    u = np.ascontiguousarray(u_vecs, dtype=f32)
    Wf = np.ascontiguousarray(W, dtype=f32)
    Wr = Wf.reshape(C, N, D)

    # iteration 1 (c uniform): s1 = colsum(u)/N, squash, t1  -- all O(B*I*C)
    s1 = u.sum(axis=1) / N  # (B, C)
    M1 = np.einsum("bc,cnd->bnd", s1, Wr)  # (B, N, D)
    o1 = M1 / np.sqrt((M1 * M1).sum(-1, keepdims=True) + EPS)
    t1 = np.einsum("bnd,cnd->bnc", o1, Wr)  # (B, N, C)

    bf = ml_dtypes.bfloat16
    # uv[b]: [128, it*256] with uv[b, p, t*C+c] = u[b, t*128+p, c]
    uv = np.ascontiguousarray(
        u.reshape(B, IT, 128, C).transpose(0, 2, 1, 3).reshape(B, 128, IT * C)
    ).astype(bf)
    if not ONDEV_T:
        # uvT[b]: [128, s*1024+i] = u[b, i, s*128+p]
        uvT = np.ascontiguousarray(
            u.transpose(0, 2, 1)
            .reshape(B, CT, 128, I)
            .transpose(0, 2, 1, 3)
            .reshape(B, 128, CT * I)
        ).astype(bf)

    shared = {
        "Wm": np.ascontiguousarray(
            Wf.reshape(CT, 128, ND).transpose(1, 0, 2)
        ).astype(f32),
        "Wt": np.ascontiguousarray(
            Wf.T.reshape(MT, 128, C).transpose(1, 0, 2)
        ).astype(f32),
        "i16": np.eye(N, dtype=f32),
        "i32": np.eye(D, dtype=f32),
        "on32": np.ones((D, 1), dtype=f32),
        "on1": np.ones((1, D), dtype=f32),
        "epsv": np.full((1, 1), EPS, dtype=f32),
    }

    in_maps = []
    for core in range(CORES):
        b0 = core * BL
        # t1T[p, s, n*BL+b] = t1[b0+b, n, s*128+p]
        t1T = np.ascontiguousarray(
            t1[b0 : b0 + BL]
            .transpose(2, 1, 0)  # (C, N, BL)
            .reshape(CT, 128, N * BL)
            .transpose(1, 0, 2)
        ).astype(bf)
        m = dict(shared)
        m["uv"] = uv[b0 : b0 + BL]
        if not ONDEV_T:
            m["uvT"] = uvT[b0 : b0 + BL]
        m["t1T"] = t1T
        in_maps.append(m)
    return in_maps


def _run(u_vecs, W, trace=False, **kw):
    if "nc" not in _CACHE:
        _CACHE["nc"] = _build_nc()
    nc = _CACHE["nc"]
    in_maps = _host_prep(u_vecs, W)
    res = run_bass_kernel_spmd(nc, in_maps, core_ids=list(range(CORES)), trace=trace, **kw)
    outs = [res.results[c]["out"] for c in range(CORES)]
    full = np.concatenate(outs, axis=0).reshape(B, N, D).astype(np.float32)
    return full, res


def kernel(u_vecs, W):
    out, _ = _run(u_vecs, W, trace=False)
    return out
